# revision 31
# baseline (speedup 1.0000x reference)
"""Fused LayerNorm + multi-head attention + out-projection for Trainium2.

Problem: x[2,2048,1024] -> LN -> QKV (16 heads, dh=64) -> softmax attention
-> out proj.  Sharded over 8 NeuronCores as batch(2) x head-groups(4)
(Megatron tensor parallel): each core handles one batch entry and 4 heads,
computing a partial out-projection; the host sums the 4 partials per batch.

Per-core dataflow (T=2048 tokens, D=1024, 4 local heads, dh=64), bf16
matmul path (psum accumulation in fp32; LN statistics in fp32):
  A/B) per 512-token chunk: LN in [tok, dim] layout (bn_stats/bn_aggr,
     fp32), normalized to bf16; xnT via the DMA crossbar transpose
     (dma_start_transpose, 14ns per 16x128 tile) instead of PE
     transposes; v natural via lhsT=xnT chunks (+ones column);
     qT,kT = (xn @ wq/wk)^T via lhsT=w chunks.  Weights are converted to
     bf16 on the host and DMA'd directly.
  C) 4 passes over (head-pair, i-half); per (jt, i2): S^T[j,i] = kT.T @ qT
     (two heads on disjoint PE row groups); exp(S/8) on ACT (psum fp32 in,
     bf16 out); O^T (+row of sums r) accumulated via (V|1).T @ expS^T.
     S matmuls run one step ahead of O matmuls so the PE never idles on
     the ACT result.  At each pass boundary the finished pass's r rows
     (packed on one partition, fp32) get one fast approx reciprocal, a
     GpSimd partition_broadcast, and DVE mults into OT -- all SBUF-side,
     nothing on the PE/ACT critical path.
  D) tail: d_one for the first token half (independent of the last pass's
     normalize), then the last normalize + second half, pipelined in
     512-col slices across PE/DVE/ACT and two DMA queues.
gamma is folded into w_qkv on the host; beta/b_out are zeros by spec
(b_out still added on the host).
"""
import numpy as np
import ml_dtypes

import concourse.bacc as bacc
import concourse.mybir as mybir
import concourse.tile as tile
from concourse import bass_utils
from concourse.masks import make_identity

F32 = mybir.dt.float32
BF16 = mybir.dt.bfloat16
AF = mybir.ActivationFunctionType
ALU = mybir.AluOpType

T = 2048          # tokens per core (one batch entry)
D = 1024          # model dim
HL = 4            # local heads per core
DH = 64           # head dim
CI = HL * DH      # local inner dim = 256
NT = T // 128     # 16 token tiles
NK = D // 128     # 8 dim chunks
LN_EPS = 1e-5
SCALE = DH ** -0.5

_NC_CACHE = {}


def _build():
    nc = bacc.Bacc("TRN2", target_bir_lowering=False, debug=False)

    x = nc.dram_tensor("x", [T, D], F32, kind="ExternalInput")
    wq = nc.dram_tensor("wq", [D, CI], BF16, kind="ExternalInput")
    wk = nc.dram_tensor("wk", [D, CI], BF16, kind="ExternalInput")
    wv = nc.dram_tensor("wv", [D, CI], BF16, kind="ExternalInput")
    wo = nc.dram_tensor("wo", [CI, D], BF16, kind="ExternalInput")
    out = nc.dram_tensor("out", [T, D], F32, kind="ExternalOutput")

    x_t = x.rearrange("(t p) d -> t p d", p=128)          # [16, 128, 1024]
    out_t = out.rearrange("(t p) d -> t p d", p=128)
    wq_t = wq.rearrange("(c p) n -> p c n", p=128)        # [128, 8, 256]
    wk_t = wk.rearrange("(c p) n -> p c n", p=128)
    wv_t = wv.rearrange("(c p) n -> p c n", p=128)
    wo_t = wo.rearrange("(c p) n -> p c n", p=128)        # [128, 2, 1024]

    with tile.TileContext(nc) as tc:
        with (
            tc.tile_pool(name="persist", bufs=1) as persist,
            tc.tile_pool(name="g_ps", bufs=1, space="PSUM") as g_ps,
        ):
            eps = persist.tile([128, 1], F32, name="eps")
            nc.vector.memset(eps, LN_EPS)
            ident_f = persist.tile([128, 128], F32, name="ident_f")
            make_identity(nc, ident_f)
            ident = persist.tile([128, 128], BF16, name="ident")
            nc.vector.tensor_copy(out=ident, in_=ident_f)

            # persistent activations / weights (all bf16)
            qkT = persist.tile([128, 4, T], BF16, name="qkT")        # 16KB/p
            vext = persist.tile([128, NT, HL, 65], BF16, name="vext")
            OT = persist.tile([128, 2, T], BF16, name="OT")          # 8KB/p
            wq_r = persist.tile([128, NK, CI], BF16, name="wq_r")
            wk_r = persist.tile([128, NK, CI], BF16, name="wk_r")
            wv_r = persist.tile([128, NK, CI], BF16, name="wv_r")
            wo_r = persist.tile([128, 2, D], BF16, name="wo_r")

            # ------------- Phase A/B: LN + transpose + QKV, ic-major -------------
            with (
                tc.tile_pool(name="ab_sb", bufs=3) as ab_sb,
                tc.tile_pool(name="ab_sm", bufs=8) as ab_sm,
            ):
                # x tile 0 split across both queues (fastest LN start);
                # weights (small in bf16) interleaved into x-stream slack;
                # deep xt buffering keeps both queues streaming
                xts = {}
                xt0 = ab_sb.tile([128, D], F32, tag="xt", name="xt", bufs=8)
                nc.sync.dma_start(xt0[:, 0:512], x_t[0][:, 0:512])
                nc.scalar.dma_start(xt0[:, 512:1024], x_t[0][:, 512:1024])
                xts[0] = xt0
                for tt in (1, 3):
                    xt1 = ab_sb.tile([128, D], F32, tag="xt", name="xt", bufs=8)
                    nc.scalar.dma_start(xt1, x_t[tt])
                    xts[tt] = xt1
                for tt in (2, 4):
                    xt2 = ab_sb.tile([128, D], F32, tag="xt", name="xt", bufs=8)
                    nc.sync.dma_start(xt2, x_t[tt])
                    xts[tt] = xt2
                nc.scalar.dma_start(wv_r, wv_t)
                nc.sync.dma_start(wq_r, wq_t)
                nc.sync.dma_start(wk_r, wk_t)

                # ones column of vext
                onev = ab_sm.tile([128, NT * HL], BF16, tag="onev")
                nc.vector.memset(onev, 1.0)
                nc.vector.tensor_copy(
                    out=vext[:, :, :, 64],
                    in_=onev.rearrange("p (t h) -> p t h", t=NT),
                )

                for ic in range(4):
                    # [d%128, tl, kc, t]: each tile's transpose lands in a
                    # contiguous [128, 8, 128] destination slice
                    xnT_ic = ab_sb.tile([128, 4, NK, 128], BF16, tag="xnTic",
                                        name="xnT_ic", bufs=2)
                    for tl in range(4):
                        tt = ic * 4 + tl
                        if tt in xts:
                            xt = xts.pop(tt)
                        else:
                            xt = ab_sb.tile([128, D], F32, tag="xt", name="xt",
                                            bufs=8)
                            (nc.scalar if tt % 2 else nc.sync).dma_start(
                                xt, x_t[tt])
                        stats = ab_sm.tile([128, 2, 6], F32, tag="stats",
                                           name="stats")
                        xr = xt.rearrange("p (c f) -> p c f", f=512)
                        for c in range(2):
                            nc.vector.bn_stats(out=stats[:, c, :], in_=xr[:, c, :])
                        mv = ab_sm.tile([128, 2], F32, tag="mv", name="mv")
                        nc.vector.bn_aggr(out=mv, in_=stats)
                        rstd = ab_sm.tile([128, 1], F32, tag="rstd", name="rstd")
                        nc.scalar.activation(out=rstd, in_=mv[:, 1:2], func=AF.Sqrt,
                                             bias=eps, scale=1.0)
                        nc.vector.reciprocal(out=rstd, in_=rstd)
                        xn = ab_sb.tile([128, D], BF16, tag="xn", name="xn",
                                        bufs=4)
                        nc.vector.tensor_scalar(out=xn, in0=xt, scalar1=mv[:, 0:1],
                                                scalar2=rstd, op0=ALU.subtract,
                                                op1=ALU.mult)
                        pt = g_ps.tile([128, NK, 128], BF16, tag="b1",
                                       name="pt", bufs=4)
                        for q in range(NK):
                            nc.tensor.transpose(
                                pt[:, q, :],
                                xn[:, q * 128:(q + 1) * 128],
                                ident)
                        nc.scalar.activation(
                            out=xnT_ic[:, tl, :, :], in_=pt, func=AF.Copy)

                        # v natural right behind this tile's transposes so
                        # the PE isn't parked waiting on later tiles' LN
                        pv = g_ps.tile([128, CI], F32, tag="b1", name="pv", bufs=4)
                        for kc in range(NK):
                            nc.tensor.matmul(
                                pv,
                                lhsT=xnT_ic[:, tl, kc, :],
                                rhs=wv_r[:, kc, :],
                                start=(kc == 0), stop=(kc == NK - 1))
                        nc.scalar.activation(
                            out=vext[:, ic * 4 + tl, :, 0:64],
                            in_=pv.rearrange("p (h d) -> p h d", h=HL),
                            func=AF.Copy)

                    # qT/kT columns for this token chunk (rhs spans all 4
                    # token tiles via a strided AP)
                    sq = [g_ps.tile([128, 1024], F32, tag="s", name=f"sq{i}",
                                    bufs=2) for i in range(2)]
                    pq = [sq[i // 2][:, (i % 2) * 512:(i % 2 + 1) * 512]
                          for i in range(4)]
                    for kc in range(NK):
                        for pc in range(4):
                            w_src = wq_r if pc < 2 else wk_r
                            off = (pc % 2) * 128
                            nc.tensor.matmul(
                                pq[pc],
                                lhsT=w_src[:, kc, off:off + 128],
                                rhs=xnT_ic[:, :, kc, :],
                                start=(kc == 0), stop=(kc == NK - 1))
                    for pc in range(4):
                        dst = qkT[:, pc, ic * 512:(ic + 1) * 512]
                        if pc % 2 == 0:
                            nc.vector.tensor_copy(out=dst, in_=pq[pc])
                        else:
                            nc.scalar.activation(out=dst, in_=pq[pc],
                                                 func=AF.Copy)

                # wo only feeds the phase-D tail; DMA it behind the x tiles
                nc.scalar.dma_start(wo_r, wo_t)

            # ---------------- Phase C: attention, 4 passes ----------------
            with (
                tc.tile_pool(name="c_exp", bufs=3) as c_exp,
                tc.tile_pool(name="c_sm", bufs=8) as c_sm,
            ):
                # packed r rows per pass: [1, 4, 512] on one partition; one
                # fast approx reciprocal per pass at the next boundary
                rqs = [c_sm.tile([1, 4, 512], F32, tag="rq", name=f"rq{p}",
                                 bufs=1)
                       for p in range(4)]

                def drip_norm(p, pr, half):
                    """normalize pass p's OT slices: reciprocal (DVE) ->
                    partition_broadcast (GpSimd, SBUF only) -> mult (DVE).
                    Touches neither PSUM nor the PE."""
                    rq = rqs[p]
                    rqf = rq.rearrange("p a b -> p (a b)")
                    nc.vector.reciprocal_approx_fast(out=rqf, in_=rqf)
                    for k in range(4):
                        hp, i2 = k // 2, k % 2
                        ic = half * 2 + i2
                        po = hp * 64
                        rb = c_sm.tile([128, 512], F32, tag="rb", name="rb",
                                       bufs=4)
                        nc.gpsimd.partition_broadcast(rb, rq[0:1, k, :])
                        sl = OT[po:po + 64, pr, ic * 512:(ic + 1) * 512]
                        nc.vector.tensor_tensor(out=sl, in0=sl,
                                                in1=rb[po:po + 64, :],
                                                op=ALU.mult)

                # passes ordered so both chunks of a token half finish early
                passes = [(0, 0), (1, 0), (0, 1), (1, 1)]
                for p, (pr, half) in enumerate(passes):
                    qc = pr                      # chunk holding both heads' q
                    kcnk = 2 + pr                # chunk holding both heads' k
                    if p > 0:
                        drip_norm(p - 1, *passes[p - 1])
                    ps_o = [g_ps.tile([65, 512], F32, tag="b1",
                                      name=f"o{i}", bufs=4)
                            for i in range(4)]   # [head parity][i2]

                    prev = None
                    steps = [(jt, i2) for jt in range(NT) for i2 in range(2)]
                    for jt, i2 in steps:
                        ic = half * 2 + i2
                        # one psum tile: [head0 chunk | head1 chunk]
                        ps_s = g_ps.tile([128, 1024], F32, tag="s",
                                         name="ps_s", bufs=2)
                        for hp in range(2):
                            po = hp * 64
                            nc.tensor.matmul(
                                ps_s[:, hp * 512:(hp + 1) * 512],
                                lhsT=qkT[po:po + 64, kcnk,
                                         jt * 128:(jt + 1) * 128],
                                rhs=qkT[po:po + 64, qc,
                                        ic * 512:(ic + 1) * 512],
                                start=True, stop=True)
                        ex = c_exp.tile([128, 1024], BF16, tag="e",
                                        name="ex", bufs=8)
                        nc.scalar.activation(out=ex, in_=ps_s,
                                             func=AF.Exp, scale=SCALE)
                        # O matmuls lag one step so the PE never waits on ACT
                        if prev is not None:
                            pjt, pi2, pex = prev
                            for hp in range(2):
                                nc.tensor.matmul(
                                    ps_o[hp * 2 + pi2],
                                    lhsT=vext[:, pjt, pr * 2 + hp, :],
                                    rhs=pex[:, hp * 512:(hp + 1) * 512],
                                    start=(pjt == 0), stop=(pjt == NT - 1),
                                    skip_group_check=True)
                        prev = (jt, i2, ex)
                    pjt, pi2, pex = prev
                    for hp in range(2):
                        nc.tensor.matmul(
                            ps_o[hp * 2 + pi2],
                            lhsT=vext[:, pjt, pr * 2 + hp, :],
                            rhs=pex[:, hp * 512:(hp + 1) * 512],
                            start=(pjt == 0), stop=(pjt == NT - 1),
                            skip_group_check=True)

                    # stash r rows + unnormalized O^T (psum->sbuf, DVE)
                    for i2 in range(2):
                        for hp in range(2):
                            ic = half * 2 + i2
                            po = hp * 64
                            nc.vector.tensor_copy(
                                out=rqs[p][0:1, hp * 2 + i2, :],
                                in_=ps_o[hp * 2 + i2][64:65, :])
                            nc.vector.tensor_copy(
                                out=OT[po:po + 64, qc,
                                       ic * 512:(ic + 1) * 512],
                                in_=ps_o[hp * 2 + i2][0:64, :])

                # ---------------- Phase D: tail ----------------
                def d_one(tt, ncn):
                    pd = g_ps.tile([128, 512], F32, tag="b1", name="pd",
                                   bufs=4)
                    for ck in range(2):
                        nc.tensor.matmul(
                            pd,
                            lhsT=OT[:, ck, tt * 128:(tt + 1) * 128],
                            rhs=wo_r[:, ck, ncn * 512:(ncn + 1) * 512],
                            start=(ck == 0), stop=(ck == 1))
                    ot = c_exp.tile([128, 512], F32, tag="ot", name="ot",
                                    bufs=6)
                    if (tt * 2 + ncn) % 2 == 0:
                        nc.scalar.activation(out=ot, in_=pd, func=AF.Copy)
                    else:
                        nc.vector.tensor_copy(out=ot, in_=pd)
                    (nc.sync if (tt * 2 + ncn) % 2 == 0
                     else nc.scalar).dma_start(
                        out_t[tt][:, ncn * 512:(ncn + 1) * 512], ot)

                # first token half doesn't depend on the last pass's norm
                for tt in range(NT // 2):
                    d_one(tt, 0)
                    d_one(tt, 1)
                drip_norm(3, *passes[3])
                for tt in range(NT // 2, NT):
                    d_one(tt, 0)
                    d_one(tt, 1)

    nc.compile()
    return nc


def make_in_maps(x, gamma, beta, w_qkv, w_out, b_out):
    """Shard full inputs into the 8 per-core input maps (batch x head-group).
    Weights are pre-converted to bf16 on the host (matmul precision)."""
    x = np.asarray(x, dtype=np.float32)
    gamma = np.asarray(gamma, dtype=np.float32)
    w_qkv = np.asarray(w_qkv, dtype=np.float32)
    w_out = np.asarray(w_out, dtype=np.float32)

    wg = (w_qkv * gamma[:, None]).astype(ml_dtypes.bfloat16)
    wo16 = w_out.astype(ml_dtypes.bfloat16)
    in_maps = []
    for core in range(8):
        b, g = core // 4, core % 4
        cs = slice(g * CI, (g + 1) * CI)
        in_maps.append({
            "x": np.ascontiguousarray(x[b]),
            "wq": np.ascontiguousarray(wg[:, 0 * 1024:1 * 1024][:, cs]),
            "wk": np.ascontiguousarray(wg[:, 1 * 1024:2 * 1024][:, cs]),
            "wv": np.ascontiguousarray(wg[:, 2 * 1024:3 * 1024][:, cs]),
            "wo": np.ascontiguousarray(wo16[cs, :]),
        })
    return in_maps


def kernel(x, gamma, beta, w_qkv, w_out, b_out):
    """Full inputs in, full output out.  Shards batch x head-groups over 8
    cores, runs the SPMD Bass kernel, and sums the partial projections."""
    if "nc" not in _NC_CACHE:
        _NC_CACHE["nc"] = _build()
    nc = _NC_CACHE["nc"]

    b_out = np.asarray(b_out, dtype=np.float32)
    in_maps = make_in_maps(x, gamma, beta, w_qkv, w_out, b_out)

    res = bass_utils.run_bass_kernel_spmd(nc, in_maps, core_ids=list(range(8)))
    parts = [r["out"] for r in res.results]
    full = np.stack([
        parts[0] + parts[1] + parts[2] + parts[3],
        parts[4] + parts[5] + parts[6] + parts[7],
    ]).astype(np.float32)
    return full + b_out


# revision 32
# speedup vs baseline: 1.0326x; 1.0326x over previous
"""Fused LayerNorm + multi-head attention + out-projection for Trainium2.

Problem: x[2,2048,1024] -> LN -> QKV (16 heads, dh=64) -> softmax attention
-> out proj.  Sharded over 8 NeuronCores as batch(2) x head-groups(4)
(Megatron tensor parallel): each core handles one batch entry and 4 heads,
computing a partial out-projection; the host sums the 4 partials per batch.

Per-core dataflow (T=2048 tokens, D=1024, 4 local heads, dh=64), bf16
matmul path (psum accumulation in fp32; LN statistics in fp32):
  A/B) per 512-token chunk: LN in [tok, dim] layout (bn_stats/bn_aggr,
     fp32), normalized to bf16; xnT via the DMA crossbar transpose
     (dma_start_transpose, 14ns per 16x128 tile) instead of PE
     transposes; v natural via lhsT=xnT chunks (+ones column);
     qT,kT = (xn @ wq/wk)^T via lhsT=w chunks.  Weights are converted to
     bf16 on the host and DMA'd directly.
  C) 4 passes over (head-pair, i-half); per (jt, i2): S^T[j,i] = kT.T @ qT
     (two heads on disjoint PE row groups); exp(S/8) on ACT (psum fp32 in,
     bf16 out); O^T (+row of sums r) accumulated via (V|1).T @ expS^T.
     S matmuls run one step ahead of O matmuls so the PE never idles on
     the ACT result.  At each pass boundary the finished pass's r rows
     (packed on one partition, fp32) get one fast approx reciprocal, a
     GpSimd partition_broadcast, and DVE mults into OT -- all SBUF-side,
     nothing on the PE/ACT critical path.
  D) tail: d_one for the first token half (independent of the last pass's
     normalize), then the last normalize + second half, pipelined in
     512-col slices across PE/DVE/ACT and two DMA queues.
gamma is folded into w_qkv on the host; beta/b_out are zeros by spec
(b_out still added on the host).
"""
import numpy as np
import ml_dtypes

import concourse.bacc as bacc
import concourse.mybir as mybir
import concourse.tile as tile
from concourse import bass_utils
from concourse.masks import make_identity

F32 = mybir.dt.float32
BF16 = mybir.dt.bfloat16
AF = mybir.ActivationFunctionType
ALU = mybir.AluOpType

T = 2048          # tokens per core (one batch entry)
D = 1024          # model dim
HL = 4            # local heads per core
DH = 64           # head dim
CI = HL * DH      # local inner dim = 256
NT = T // 128     # 16 token tiles
NK = D // 128     # 8 dim chunks
LN_EPS = 1e-5
SCALE = DH ** -0.5

_NC_CACHE = {}


def _build():
    nc = bacc.Bacc("TRN2", target_bir_lowering=False, debug=False)

    x = nc.dram_tensor("x", [T, D], F32, kind="ExternalInput")
    wq = nc.dram_tensor("wq", [D, CI], BF16, kind="ExternalInput")
    wk = nc.dram_tensor("wk", [D, CI], BF16, kind="ExternalInput")
    wv = nc.dram_tensor("wv", [D, CI], BF16, kind="ExternalInput")
    wo = nc.dram_tensor("wo", [CI, D], BF16, kind="ExternalInput")
    out = nc.dram_tensor("out", [T, D], F32, kind="ExternalOutput")

    x_t = x.rearrange("(t p) d -> t p d", p=128)          # [16, 128, 1024]
    out_t = out.rearrange("(t p) d -> t p d", p=128)
    wq_t = wq.rearrange("(c p) n -> p c n", p=128)        # [128, 8, 256]
    wk_t = wk.rearrange("(c p) n -> p c n", p=128)
    wv_t = wv.rearrange("(c p) n -> p c n", p=128)
    wo_t = wo.rearrange("(c p) n -> p c n", p=128)        # [128, 2, 1024]

    with tile.TileContext(nc) as tc:
        with (
            tc.tile_pool(name="persist", bufs=1) as persist,
            tc.tile_pool(name="g_ps", bufs=1, space="PSUM") as g_ps,
        ):
            eps = persist.tile([128, 1], F32, name="eps")
            nc.vector.memset(eps, LN_EPS)
            ident_f = persist.tile([128, 128], F32, name="ident_f")
            make_identity(nc, ident_f)
            ident = persist.tile([128, 128], BF16, name="ident")
            nc.vector.tensor_copy(out=ident, in_=ident_f)

            # persistent activations / weights (all bf16)
            qkT = persist.tile([128, 4, T], BF16, name="qkT")        # 16KB/p
            vext = persist.tile([128, NT, HL, 65], BF16, name="vext")
            OT = persist.tile([128, 2, T], BF16, name="OT")          # 8KB/p
            wq_r = persist.tile([128, NK, CI], BF16, name="wq_r")
            wk_r = persist.tile([128, NK, CI], BF16, name="wk_r")
            wv_r = persist.tile([128, NK, CI], BF16, name="wv_r")
            wo_r = persist.tile([128, 2, D], BF16, name="wo_r")

            # ------------- Phase A/B: LN + transpose + QKV, ic-major -------------
            with (
                tc.tile_pool(name="ab_sb", bufs=3) as ab_sb,
                tc.tile_pool(name="ab_sm", bufs=8) as ab_sm,
            ):
                # x tile 0 split across both queues (fastest LN start);
                # weights (small in bf16) interleaved into x-stream slack;
                # deep xt buffering keeps both queues streaming
                xts = {}
                xt0 = ab_sb.tile([128, D], F32, tag="xt", name="xt", bufs=8)
                nc.sync.dma_start(xt0[:, 0:512], x_t[0][:, 0:512])
                nc.scalar.dma_start(xt0[:, 512:1024], x_t[0][:, 512:1024])
                xts[0] = xt0
                for tt in (1, 3):
                    xt1 = ab_sb.tile([128, D], F32, tag="xt", name="xt", bufs=8)
                    nc.scalar.dma_start(xt1, x_t[tt])
                    xts[tt] = xt1
                for tt in (2, 4):
                    xt2 = ab_sb.tile([128, D], F32, tag="xt", name="xt", bufs=8)
                    nc.sync.dma_start(xt2, x_t[tt])
                    xts[tt] = xt2
                nc.scalar.dma_start(wv_r, wv_t)
                nc.sync.dma_start(wq_r, wq_t)
                nc.sync.dma_start(wk_r, wk_t)

                # ones column of vext
                onev = ab_sm.tile([128, NT * HL], BF16, tag="onev")
                nc.vector.memset(onev, 1.0)
                nc.vector.tensor_copy(
                    out=vext[:, :, :, 64],
                    in_=onev.rearrange("p (t h) -> p t h", t=NT),
                )

                for ic in range(4):
                    # [d%128, tl, kc, t]: each tile's transpose lands in a
                    # contiguous [128, 8, 128] destination slice
                    xnT_ic = ab_sb.tile([128, 4, NK, 128], BF16, tag="xnTic",
                                        name="xnT_ic", bufs=2)
                    for tl in range(4):
                        tt = ic * 4 + tl
                        if tt in xts:
                            xt = xts.pop(tt)
                        else:
                            xt = ab_sb.tile([128, D], F32, tag="xt", name="xt",
                                            bufs=8)
                            (nc.scalar if tt % 2 else nc.sync).dma_start(
                                xt, x_t[tt])
                        stats = ab_sm.tile([128, 2, 6], F32, tag="stats",
                                           name="stats")
                        xr = xt.rearrange("p (c f) -> p c f", f=512)
                        for c in range(2):
                            nc.vector.bn_stats(out=stats[:, c, :], in_=xr[:, c, :])
                        mv = ab_sm.tile([128, 2], F32, tag="mv", name="mv")
                        nc.vector.bn_aggr(out=mv, in_=stats)
                        rstd = ab_sm.tile([128, 1], F32, tag="rstd", name="rstd")
                        nc.scalar.activation(out=rstd, in_=mv[:, 1:2], func=AF.Sqrt,
                                             bias=eps, scale=1.0)
                        nc.vector.reciprocal(out=rstd, in_=rstd)
                        xn = ab_sb.tile([128, D], BF16, tag="xn", name="xn",
                                        bufs=4)
                        nc.vector.tensor_scalar(out=xn, in0=xt, scalar1=mv[:, 0:1],
                                                scalar2=rstd, op0=ALU.subtract,
                                                op1=ALU.mult)
                        for kc4 in range(NK // 4):
                            pt = g_ps.tile([128, 4, 128], BF16, tag="b1",
                                           name="pt", bufs=4)
                            for q in range(4):
                                nc.tensor.transpose(
                                    pt[:, q, :],
                                    xn[:, (kc4 * 4 + q) * 128:(kc4 * 4 + q + 1) * 128],
                                    ident)
                            nc.scalar.activation(
                                out=xnT_ic[:, tl, kc4 * 4:kc4 * 4 + 4, :],
                                in_=pt, func=AF.Copy)

                    # v natural for these 4 token tiles
                    for tl in range(4):
                        pv = g_ps.tile([128, CI], F32, tag="b1", name="pv", bufs=4)
                        for kc in range(NK):
                            nc.tensor.matmul(
                                pv,
                                lhsT=xnT_ic[:, tl, kc, :],
                                rhs=wv_r[:, kc, :],
                                start=(kc == 0), stop=(kc == NK - 1))
                        nc.vector.tensor_copy(
                            out=vext[:, ic * 4 + tl, :, 0:64],
                            in_=pv.rearrange("p (h d) -> p h d", h=HL))

                    # qT/kT columns for this token chunk (rhs spans all 4
                    # token tiles via a strided AP)
                    sq = [g_ps.tile([128, 1024], F32, tag="s", name=f"sq{i}",
                                    bufs=2) for i in range(2)]
                    pq = [sq[i // 2][:, (i % 2) * 512:(i % 2 + 1) * 512]
                          for i in range(4)]
                    for kc in range(NK):
                        for pc in range(4):
                            w_src = wq_r if pc < 2 else wk_r
                            off = (pc % 2) * 128
                            nc.tensor.matmul(
                                pq[pc],
                                lhsT=w_src[:, kc, off:off + 128],
                                rhs=xnT_ic[:, :, kc, :],
                                start=(kc == 0), stop=(kc == NK - 1))
                    for pc in range(4):
                        dst = qkT[:, pc, ic * 512:(ic + 1) * 512]
                        if pc % 2 == 0:
                            nc.vector.tensor_copy(out=dst, in_=pq[pc])
                        else:
                            nc.scalar.activation(out=dst, in_=pq[pc],
                                                 func=AF.Copy)

                # wo only feeds the phase-D tail; DMA it behind the x tiles
                nc.scalar.dma_start(wo_r, wo_t)

            # ---------------- Phase C: attention, 4 passes ----------------
            with (
                tc.tile_pool(name="c_exp", bufs=3) as c_exp,
                tc.tile_pool(name="c_sm", bufs=8) as c_sm,
            ):
                # packed r rows per pass: [1, 4, 512] on one partition; one
                # fast approx reciprocal per pass at the next boundary
                rqs = [c_sm.tile([1, 4, 512], F32, tag="rq", name=f"rq{p}",
                                 bufs=1)
                       for p in range(4)]

                def drip_norm(p, pr, half):
                    """normalize pass p's OT slices: reciprocal (DVE) ->
                    partition_broadcast (GpSimd, SBUF only) -> mult (DVE).
                    Touches neither PSUM nor the PE."""
                    rq = rqs[p]
                    rqf = rq.rearrange("p a b -> p (a b)")
                    nc.vector.reciprocal_approx_fast(out=rqf, in_=rqf)
                    for k in range(4):
                        hp, i2 = k // 2, k % 2
                        ic = half * 2 + i2
                        po = hp * 64
                        rb = c_sm.tile([128, 512], F32, tag="rb", name="rb",
                                       bufs=4)
                        nc.gpsimd.partition_broadcast(rb, rq[0:1, k, :])
                        sl = OT[po:po + 64, pr, ic * 512:(ic + 1) * 512]
                        nc.vector.tensor_tensor(out=sl, in0=sl,
                                                in1=rb[po:po + 64, :],
                                                op=ALU.mult)

                # passes ordered so both chunks of a token half finish early
                passes = [(0, 0), (1, 0), (0, 1), (1, 1)]
                for p, (pr, half) in enumerate(passes):
                    qc = pr                      # chunk holding both heads' q
                    kcnk = 2 + pr                # chunk holding both heads' k
                    if p > 0:
                        drip_norm(p - 1, *passes[p - 1])
                    ps_o = [g_ps.tile([65, 512], F32, tag="b1",
                                      name=f"o{i}", bufs=4)
                            for i in range(4)]   # [head parity][i2]

                    prev = None
                    steps = [(jt, i2) for jt in range(NT) for i2 in range(2)]
                    for jt, i2 in steps:
                        ic = half * 2 + i2
                        # one psum tile: [head0 chunk | head1 chunk]
                        ps_s = g_ps.tile([128, 1024], F32, tag="s",
                                         name="ps_s", bufs=2)
                        for hp in range(2):
                            po = hp * 64
                            nc.tensor.matmul(
                                ps_s[:, hp * 512:(hp + 1) * 512],
                                lhsT=qkT[po:po + 64, kcnk,
                                         jt * 128:(jt + 1) * 128],
                                rhs=qkT[po:po + 64, qc,
                                        ic * 512:(ic + 1) * 512],
                                start=True, stop=True)
                        ex = c_exp.tile([128, 1024], BF16, tag="e",
                                        name="ex", bufs=8)
                        nc.scalar.activation(out=ex, in_=ps_s,
                                             func=AF.Exp, scale=SCALE)
                        # O matmuls lag one step so the PE never waits on ACT
                        if prev is not None:
                            pjt, pi2, pex = prev
                            for hp in range(2):
                                nc.tensor.matmul(
                                    ps_o[hp * 2 + pi2],
                                    lhsT=vext[:, pjt, pr * 2 + hp, :],
                                    rhs=pex[:, hp * 512:(hp + 1) * 512],
                                    start=(pjt == 0), stop=(pjt == NT - 1),
                                    skip_group_check=True)
                        prev = (jt, i2, ex)
                    pjt, pi2, pex = prev
                    for hp in range(2):
                        nc.tensor.matmul(
                            ps_o[hp * 2 + pi2],
                            lhsT=vext[:, pjt, pr * 2 + hp, :],
                            rhs=pex[:, hp * 512:(hp + 1) * 512],
                            start=(pjt == 0), stop=(pjt == NT - 1),
                            skip_group_check=True)

                    # stash r rows + unnormalized O^T (psum->sbuf, DVE)
                    for i2 in range(2):
                        for hp in range(2):
                            ic = half * 2 + i2
                            po = hp * 64
                            nc.vector.tensor_copy(
                                out=rqs[p][0:1, hp * 2 + i2, :],
                                in_=ps_o[hp * 2 + i2][64:65, :])
                            nc.vector.tensor_copy(
                                out=OT[po:po + 64, qc,
                                       ic * 512:(ic + 1) * 512],
                                in_=ps_o[hp * 2 + i2][0:64, :])

                # ---------------- Phase D: tail ----------------
                def d_one(tt, ncn):
                    pd = g_ps.tile([128, 512], F32, tag="b1", name="pd",
                                   bufs=4)
                    for ck in range(2):
                        nc.tensor.matmul(
                            pd,
                            lhsT=OT[:, ck, tt * 128:(tt + 1) * 128],
                            rhs=wo_r[:, ck, ncn * 512:(ncn + 1) * 512],
                            start=(ck == 0), stop=(ck == 1))
                    ot = c_exp.tile([128, 512], F32, tag="ot", name="ot",
                                    bufs=6)
                    if (tt * 2 + ncn) % 2 == 0:
                        nc.scalar.activation(out=ot, in_=pd, func=AF.Copy)
                    else:
                        nc.vector.tensor_copy(out=ot, in_=pd)
                    (nc.sync if (tt * 2 + ncn) % 2 == 0
                     else nc.scalar).dma_start(
                        out_t[tt][:, ncn * 512:(ncn + 1) * 512], ot)

                # first token half doesn't depend on the last pass's norm
                for tt in range(NT // 2):
                    d_one(tt, 0)
                    d_one(tt, 1)
                drip_norm(3, *passes[3])
                for tt in range(NT // 2, NT):
                    d_one(tt, 0)
                    d_one(tt, 1)

    nc.compile()
    return nc


def make_in_maps(x, gamma, beta, w_qkv, w_out, b_out):
    """Shard full inputs into the 8 per-core input maps (batch x head-group).
    Weights are pre-converted to bf16 on the host (matmul precision)."""
    x = np.asarray(x, dtype=np.float32)
    gamma = np.asarray(gamma, dtype=np.float32)
    w_qkv = np.asarray(w_qkv, dtype=np.float32)
    w_out = np.asarray(w_out, dtype=np.float32)

    wg = (w_qkv * gamma[:, None]).astype(ml_dtypes.bfloat16)
    wo16 = w_out.astype(ml_dtypes.bfloat16)
    in_maps = []
    for core in range(8):
        b, g = core // 4, core % 4
        cs = slice(g * CI, (g + 1) * CI)
        in_maps.append({
            "x": np.ascontiguousarray(x[b]),
            "wq": np.ascontiguousarray(wg[:, 0 * 1024:1 * 1024][:, cs]),
            "wk": np.ascontiguousarray(wg[:, 1 * 1024:2 * 1024][:, cs]),
            "wv": np.ascontiguousarray(wg[:, 2 * 1024:3 * 1024][:, cs]),
            "wo": np.ascontiguousarray(wo16[cs, :]),
        })
    return in_maps


def kernel(x, gamma, beta, w_qkv, w_out, b_out):
    """Full inputs in, full output out.  Shards batch x head-groups over 8
    cores, runs the SPMD Bass kernel, and sums the partial projections."""
    if "nc" not in _NC_CACHE:
        _NC_CACHE["nc"] = _build()
    nc = _NC_CACHE["nc"]

    b_out = np.asarray(b_out, dtype=np.float32)
    in_maps = make_in_maps(x, gamma, beta, w_qkv, w_out, b_out)

    res = bass_utils.run_bass_kernel_spmd(nc, in_maps, core_ids=list(range(8)))
    parts = [r["out"] for r in res.results]
    full = np.stack([
        parts[0] + parts[1] + parts[2] + parts[3],
        parts[4] + parts[5] + parts[6] + parts[7],
    ]).astype(np.float32)
    return full + b_out


# revision 33
# speedup vs baseline: 1.0363x; 1.0036x over previous
"""Fused LayerNorm + multi-head attention + out-projection for Trainium2.

Problem: x[2,2048,1024] -> LN -> QKV (16 heads, dh=64) -> softmax attention
-> out proj.  Sharded over 8 NeuronCores as batch(2) x head-groups(4)
(Megatron tensor parallel): each core handles one batch entry and 4 heads,
computing a partial out-projection; the host sums the 4 partials per batch.

Per-core dataflow (T=2048 tokens, D=1024, 4 local heads, dh=64), bf16
matmul path (psum accumulation in fp32; LN statistics in fp32):
  A/B) per 512-token chunk: LN in [tok, dim] layout (bn_stats/bn_aggr,
     fp32), normalized to bf16; PE-transpose to xnT [d, tok] (bf16,
     1 cyc/row) with psum->sbuf stashes on the ACT engine; v natural via
     lhsT=xnT chunks (+ones column); qT,kT = (xn @ wq/wk)^T via lhsT=w
     chunks.  Weights are converted to bf16 on the host and DMA'd
     directly (no on-chip cast pass).  (Note: the DMA crossbar transpose
     would be cheaper but corrupts data when >2 cores run it
     concurrently on this runtime.)
  C) 4 passes over (head-pair, i-half); per (jt, i2): S^T[j,i] = kT.T @ qT
     (two heads on disjoint PE row groups); exp(S/8) on ACT (psum fp32 in,
     bf16 out); O^T (+row of sums r) accumulated via (V|1).T @ expS^T.
     S matmuls run one step ahead of O matmuls so the PE never idles on
     the ACT result.  At each pass boundary the finished pass's r rows
     (packed on one partition, fp32) get one fast approx reciprocal, a
     GpSimd partition_broadcast, and DVE mults into OT -- all SBUF-side,
     nothing on the PE/ACT critical path.
  D) tail: d_one for the first token half (independent of the last pass's
     normalize), then the last normalize + second half, pipelined in
     512-col slices across PE/DVE/ACT and two DMA queues.
gamma is folded into w_qkv on the host; beta/b_out are zeros by spec
(b_out still added on the host).
"""
import numpy as np
import ml_dtypes

import concourse.bacc as bacc
import concourse.mybir as mybir
import concourse.tile as tile
from concourse import bass_utils
from concourse.masks import make_identity

F32 = mybir.dt.float32
BF16 = mybir.dt.bfloat16
AF = mybir.ActivationFunctionType
ALU = mybir.AluOpType

T = 2048          # tokens per core (one batch entry)
D = 1024          # model dim
HL = 4            # local heads per core
DH = 64           # head dim
CI = HL * DH      # local inner dim = 256
NT = T // 128     # 16 token tiles
NK = D // 128     # 8 dim chunks
LN_EPS = 1e-5
SCALE = DH ** -0.5

_NC_CACHE = {}


def _build():
    nc = bacc.Bacc("TRN2", target_bir_lowering=False, debug=False)

    x = nc.dram_tensor("x", [T, D], F32, kind="ExternalInput")
    wq = nc.dram_tensor("wq", [D, CI], BF16, kind="ExternalInput")
    wk = nc.dram_tensor("wk", [D, CI], BF16, kind="ExternalInput")
    wv = nc.dram_tensor("wv", [D, CI], BF16, kind="ExternalInput")
    wo = nc.dram_tensor("wo", [CI, D], BF16, kind="ExternalInput")
    out = nc.dram_tensor("out", [T, D], F32, kind="ExternalOutput")

    x_t = x.rearrange("(t p) d -> t p d", p=128)          # [16, 128, 1024]
    out_t = out.rearrange("(t p) d -> t p d", p=128)
    wq_t = wq.rearrange("(c p) n -> p c n", p=128)        # [128, 8, 256]
    wk_t = wk.rearrange("(c p) n -> p c n", p=128)
    wv_t = wv.rearrange("(c p) n -> p c n", p=128)
    wo_t = wo.rearrange("(c p) n -> p c n", p=128)        # [128, 2, 1024]

    with tile.TileContext(nc) as tc:
        with (
            tc.tile_pool(name="persist", bufs=1) as persist,
            tc.tile_pool(name="g_ps", bufs=1, space="PSUM") as g_ps,
        ):
            eps = persist.tile([128, 1], F32, name="eps")
            nc.vector.memset(eps, LN_EPS)
            ident_f = persist.tile([128, 128], F32, name="ident_f")
            make_identity(nc, ident_f)
            ident = persist.tile([128, 128], BF16, name="ident")
            nc.vector.tensor_copy(out=ident, in_=ident_f)

            # persistent activations / weights (all bf16)
            qkT = persist.tile([128, 4, T], BF16, name="qkT")        # 16KB/p
            vext = persist.tile([128, NT, HL, 65], BF16, name="vext")
            OT = persist.tile([128, 2, T], BF16, name="OT")          # 8KB/p
            wq_r = persist.tile([128, NK, CI], BF16, name="wq_r")
            wk_r = persist.tile([128, NK, CI], BF16, name="wk_r")
            wv_r = persist.tile([128, NK, CI], BF16, name="wv_r")
            wo_r = persist.tile([128, 2, D], BF16, name="wo_r")

            # ------------- Phase A/B: LN + transpose + QKV, ic-major -------------
            with (
                tc.tile_pool(name="ab_sb", bufs=3) as ab_sb,
                tc.tile_pool(name="ab_sm", bufs=8) as ab_sm,
            ):
                # x tile 0 split across both queues (fastest LN start);
                # weights (small in bf16) interleaved into x-stream slack;
                # deep xt buffering keeps both queues streaming
                xts = {}
                xt0 = ab_sb.tile([128, D], F32, tag="xt", name="xt", bufs=8)
                nc.sync.dma_start(xt0[:, 0:512], x_t[0][:, 0:512])
                nc.scalar.dma_start(xt0[:, 512:1024], x_t[0][:, 512:1024])
                xts[0] = xt0
                for tt in (1, 3):
                    xt1 = ab_sb.tile([128, D], F32, tag="xt", name="xt", bufs=8)
                    nc.scalar.dma_start(xt1, x_t[tt])
                    xts[tt] = xt1
                for tt in (2, 4):
                    xt2 = ab_sb.tile([128, D], F32, tag="xt", name="xt", bufs=8)
                    nc.sync.dma_start(xt2, x_t[tt])
                    xts[tt] = xt2
                nc.scalar.dma_start(wv_r, wv_t)
                nc.sync.dma_start(wq_r, wq_t)
                nc.sync.dma_start(wk_r, wk_t)

                # ones column of vext
                onev = ab_sm.tile([128, NT * HL], BF16, tag="onev")
                nc.vector.memset(onev, 1.0)
                nc.vector.tensor_copy(
                    out=vext[:, :, :, 64],
                    in_=onev.rearrange("p (t h) -> p t h", t=NT),
                )

                for ic in range(4):
                    # [d%128, tl, kc, t]: each tile's transpose lands in a
                    # contiguous [128, 8, 128] destination slice
                    xnT_ic = ab_sb.tile([128, 4, NK, 128], BF16, tag="xnTic",
                                        name="xnT_ic", bufs=2)
                    for tl in range(4):
                        tt = ic * 4 + tl
                        if tt in xts:
                            xt = xts.pop(tt)
                        else:
                            xt = ab_sb.tile([128, D], F32, tag="xt", name="xt",
                                            bufs=8)
                            (nc.scalar if tt % 2 else nc.sync).dma_start(
                                xt, x_t[tt])
                        stats = ab_sm.tile([128, 2, 6], F32, tag="stats",
                                           name="stats")
                        xr = xt.rearrange("p (c f) -> p c f", f=512)
                        for c in range(2):
                            nc.vector.bn_stats(out=stats[:, c, :], in_=xr[:, c, :])
                        mv = ab_sm.tile([128, 2], F32, tag="mv", name="mv")
                        nc.vector.bn_aggr(out=mv, in_=stats)
                        rstd = ab_sm.tile([128, 1], F32, tag="rstd", name="rstd")
                        nc.scalar.activation(out=rstd, in_=mv[:, 1:2], func=AF.Sqrt,
                                             bias=eps, scale=1.0)
                        nc.vector.reciprocal(out=rstd, in_=rstd)
                        xn = ab_sb.tile([128, D], BF16, tag="xn", name="xn",
                                        bufs=4)
                        nc.vector.tensor_scalar(out=xn, in0=xt, scalar1=mv[:, 0:1],
                                                scalar2=rstd, op0=ALU.subtract,
                                                op1=ALU.mult)
                        for kc4 in range(NK // 4):
                            pt = g_ps.tile([128, 4, 128], BF16, tag="b1",
                                           name="pt", bufs=4)
                            for q in range(4):
                                nc.tensor.transpose(
                                    pt[:, q, :],
                                    xn[:, (kc4 * 4 + q) * 128:(kc4 * 4 + q + 1) * 128],
                                    ident)
                            nc.scalar.activation(
                                out=xnT_ic[:, tl, kc4 * 4:kc4 * 4 + 4, :],
                                in_=pt, func=AF.Copy)

                    # v natural for these 4 token tiles
                    for tl in range(4):
                        pv = g_ps.tile([128, CI], F32, tag="b1", name="pv", bufs=4)
                        for kc in range(NK):
                            nc.tensor.matmul(
                                pv,
                                lhsT=xnT_ic[:, tl, kc, :],
                                rhs=wv_r[:, kc, :],
                                start=(kc == 0), stop=(kc == NK - 1))
                        nc.vector.tensor_copy(
                            out=vext[:, ic * 4 + tl, :, 0:64],
                            in_=pv.rearrange("p (h d) -> p h d", h=HL))

                    # qT/kT columns for this token chunk (rhs spans all 4
                    # token tiles via a strided AP)
                    sq = [g_ps.tile([128, 1024], F32, tag="s", name=f"sq{i}",
                                    bufs=2) for i in range(2)]
                    pq = [sq[i // 2][:, (i % 2) * 512:(i % 2 + 1) * 512]
                          for i in range(4)]
                    for kc in range(NK):
                        for pc in range(4):
                            w_src = wq_r if pc < 2 else wk_r
                            off = (pc % 2) * 128
                            nc.tensor.matmul(
                                pq[pc],
                                lhsT=w_src[:, kc, off:off + 128],
                                rhs=xnT_ic[:, :, kc, :],
                                start=(kc == 0), stop=(kc == NK - 1))
                    for pc in range(4):
                        dst = qkT[:, pc, ic * 512:(ic + 1) * 512]
                        if pc % 2 == 0:
                            nc.vector.tensor_copy(out=dst, in_=pq[pc])
                        else:
                            nc.scalar.activation(out=dst, in_=pq[pc],
                                                 func=AF.Copy)

                # wo only feeds the phase-D tail; DMA it behind the x tiles
                nc.scalar.dma_start(wo_r, wo_t)

            # ---------------- Phase C: attention, 4 passes ----------------
            with (
                tc.tile_pool(name="c_exp", bufs=3) as c_exp,
                tc.tile_pool(name="c_sm", bufs=8) as c_sm,
            ):
                # packed r rows per pass: [1, 4, 512] on one partition; one
                # fast approx reciprocal per pass at the next boundary
                rqs = [c_sm.tile([1, 4, 512], F32, tag="rq", name=f"rq{p}",
                                 bufs=1)
                       for p in range(4)]

                def drip_norm(p, pr, half):
                    """normalize pass p's OT slices: reciprocal (DVE) ->
                    partition_broadcast (GpSimd, SBUF only) -> mult (DVE).
                    Touches neither PSUM nor the PE."""
                    rq = rqs[p]
                    rqf = rq.rearrange("p a b -> p (a b)")
                    nc.vector.reciprocal_approx_fast(out=rqf, in_=rqf)
                    for k in range(4):
                        hp, i2 = k // 2, k % 2
                        ic = half * 2 + i2
                        po = hp * 64
                        rb = c_sm.tile([128, 512], F32, tag="rb", name="rb",
                                       bufs=4)
                        nc.gpsimd.partition_broadcast(rb, rq[0:1, k, :])
                        sl = OT[po:po + 64, pr, ic * 512:(ic + 1) * 512]
                        nc.vector.tensor_tensor(out=sl, in0=sl,
                                                in1=rb[po:po + 64, :],
                                                op=ALU.mult)

                # passes ordered so both chunks of a token half finish early
                passes = [(0, 0), (1, 0), (0, 1), (1, 1)]
                for p, (pr, half) in enumerate(passes):
                    qc = pr                      # chunk holding both heads' q
                    kcnk = 2 + pr                # chunk holding both heads' k
                    if p > 0:
                        drip_norm(p - 1, *passes[p - 1])
                    ps_o = [g_ps.tile([65, 512], F32, tag="b1",
                                      name=f"o{i}", bufs=4)
                            for i in range(4)]   # [head parity][i2]

                    prev = None
                    steps = [(jt, i2) for jt in range(NT) for i2 in range(2)]
                    for jt, i2 in steps:
                        ic = half * 2 + i2
                        # one psum tile: [head0 chunk | head1 chunk]
                        ps_s = g_ps.tile([128, 1024], F32, tag="s",
                                         name="ps_s", bufs=2)
                        for hp in range(2):
                            po = hp * 64
                            nc.tensor.matmul(
                                ps_s[:, hp * 512:(hp + 1) * 512],
                                lhsT=qkT[po:po + 64, kcnk,
                                         jt * 128:(jt + 1) * 128],
                                rhs=qkT[po:po + 64, qc,
                                        ic * 512:(ic + 1) * 512],
                                start=True, stop=True)
                        ex = c_exp.tile([128, 1024], BF16, tag="e",
                                        name="ex", bufs=8)
                        nc.scalar.activation(out=ex, in_=ps_s,
                                             func=AF.Exp, scale=SCALE)
                        # O matmuls lag one step so the PE never waits on ACT
                        if prev is not None:
                            pjt, pi2, pex = prev
                            for hp in range(2):
                                nc.tensor.matmul(
                                    ps_o[hp * 2 + pi2],
                                    lhsT=vext[:, pjt, pr * 2 + hp, :],
                                    rhs=pex[:, hp * 512:(hp + 1) * 512],
                                    start=(pjt == 0), stop=(pjt == NT - 1),
                                    skip_group_check=True)
                        prev = (jt, i2, ex)
                    pjt, pi2, pex = prev
                    for hp in range(2):
                        nc.tensor.matmul(
                            ps_o[hp * 2 + pi2],
                            lhsT=vext[:, pjt, pr * 2 + hp, :],
                            rhs=pex[:, hp * 512:(hp + 1) * 512],
                            start=(pjt == 0), stop=(pjt == NT - 1),
                            skip_group_check=True)

                    # stash r rows + unnormalized O^T (psum->sbuf, DVE)
                    for i2 in range(2):
                        for hp in range(2):
                            ic = half * 2 + i2
                            po = hp * 64
                            nc.vector.tensor_copy(
                                out=rqs[p][0:1, hp * 2 + i2, :],
                                in_=ps_o[hp * 2 + i2][64:65, :])
                            nc.vector.tensor_copy(
                                out=OT[po:po + 64, qc,
                                       ic * 512:(ic + 1) * 512],
                                in_=ps_o[hp * 2 + i2][0:64, :])

                # ---------------- Phase D: tail ----------------
                def d_one(tt, ncn):
                    pd = g_ps.tile([128, 512], F32, tag="b1", name="pd",
                                   bufs=4)
                    for ck in range(2):
                        nc.tensor.matmul(
                            pd,
                            lhsT=OT[:, ck, tt * 128:(tt + 1) * 128],
                            rhs=wo_r[:, ck, ncn * 512:(ncn + 1) * 512],
                            start=(ck == 0), stop=(ck == 1))
                    ot = c_exp.tile([128, 512], F32, tag="ot", name="ot",
                                    bufs=6)
                    if (tt * 2 + ncn) % 2 == 0:
                        nc.scalar.activation(out=ot, in_=pd, func=AF.Copy)
                    else:
                        nc.vector.tensor_copy(out=ot, in_=pd)
                    (nc.sync if (tt * 2 + ncn) % 2 == 0
                     else nc.scalar).dma_start(
                        out_t[tt][:, ncn * 512:(ncn + 1) * 512], ot)

                # first token half doesn't depend on the last pass's norm
                for tt in range(NT // 2):
                    d_one(tt, 0)
                    d_one(tt, 1)
                drip_norm(3, *passes[3])
                for tt in range(NT // 2, NT):
                    d_one(tt, 0)
                    d_one(tt, 1)

    nc.compile()
    return nc


def make_in_maps(x, gamma, beta, w_qkv, w_out, b_out):
    """Shard full inputs into the 8 per-core input maps (batch x head-group).
    Weights are pre-converted to bf16 on the host (matmul precision)."""
    x = np.asarray(x, dtype=np.float32)
    gamma = np.asarray(gamma, dtype=np.float32)
    w_qkv = np.asarray(w_qkv, dtype=np.float32)
    w_out = np.asarray(w_out, dtype=np.float32)

    wg = (w_qkv * gamma[:, None]).astype(ml_dtypes.bfloat16)
    wo16 = w_out.astype(ml_dtypes.bfloat16)
    in_maps = []
    for core in range(8):
        b, g = core // 4, core % 4
        cs = slice(g * CI, (g + 1) * CI)
        in_maps.append({
            "x": np.ascontiguousarray(x[b]),
            "wq": np.ascontiguousarray(wg[:, 0 * 1024:1 * 1024][:, cs]),
            "wk": np.ascontiguousarray(wg[:, 1 * 1024:2 * 1024][:, cs]),
            "wv": np.ascontiguousarray(wg[:, 2 * 1024:3 * 1024][:, cs]),
            "wo": np.ascontiguousarray(wo16[cs, :]),
        })
    return in_maps


def kernel(x, gamma, beta, w_qkv, w_out, b_out):
    """Full inputs in, full output out.  Shards batch x head-groups over 8
    cores, runs the SPMD Bass kernel, and sums the partial projections."""
    if "nc" not in _NC_CACHE:
        _NC_CACHE["nc"] = _build()
    nc = _NC_CACHE["nc"]

    b_out = np.asarray(b_out, dtype=np.float32)
    in_maps = make_in_maps(x, gamma, beta, w_qkv, w_out, b_out)

    res = bass_utils.run_bass_kernel_spmd(nc, in_maps, core_ids=list(range(8)))
    parts = [r["out"] for r in res.results]
    full = np.stack([
        parts[0] + parts[1] + parts[2] + parts[3],
        parts[4] + parts[5] + parts[6] + parts[7],
    ]).astype(np.float32)
    return full + b_out


# revision 35
# speedup vs baseline: 1.0533x; 1.0165x over previous
"""Fused LayerNorm + multi-head attention + out-projection for Trainium2.

Problem: x[2,2048,1024] -> LN -> QKV (16 heads, dh=64) -> softmax attention
-> out proj.  Sharded over 8 NeuronCores as batch(2) x head-groups(4)
(Megatron tensor parallel): each core handles one batch entry and 4 heads,
computing a partial out-projection; the host sums the 4 partials per batch.

Per-core dataflow (T=2048 tokens, D=1024, 4 local heads, dh=64), bf16
matmul path (psum accumulation in fp32; LN statistics in fp32):
  A/B) per 512-token chunk: LN in [tok, dim] layout (bn_stats/bn_aggr,
     fp32), normalized to bf16; PE-transpose to xnT [d, tok] (bf16,
     1 cyc/row) with psum->sbuf stashes on the ACT engine; v natural via
     lhsT=xnT chunks (+ones column); qT,kT = (xn @ wq/wk)^T via lhsT=w
     chunks.  Weights are converted to bf16 on the host and DMA'd
     directly (no on-chip cast pass).  (Note: the DMA crossbar transpose
     would be cheaper but corrupts data when >2 cores run it
     concurrently on this runtime.)
  C) 4 passes over (head-pair, i-half); per (jt, i2): S^T[j,i] = kT.T @ qT
     (two heads on disjoint PE row groups); exp(S/8) on ACT (psum fp32 in,
     bf16 out); O^T (+row of sums r) accumulated via (V|1).T @ expS^T.
     S matmuls run one step ahead of O matmuls so the PE never idles on
     the ACT result.  At each pass boundary the finished pass's r rows
     (packed on one partition, fp32) get one fast approx reciprocal, a
     GpSimd partition_broadcast, and DVE mults into OT -- all SBUF-side,
     nothing on the PE/ACT critical path.
  D) tail: d_one for the first token half (independent of the last pass's
     normalize), then the last normalize + second half, pipelined in
     512-col slices across PE/DVE/ACT and two DMA queues.
gamma is folded into w_qkv on the host; beta/b_out are zeros by spec
(b_out still added on the host).
"""
import numpy as np
import ml_dtypes

import concourse.bacc as bacc
import concourse.mybir as mybir
import concourse.tile as tile
from concourse import bass_utils
from concourse.masks import make_identity

F32 = mybir.dt.float32
BF16 = mybir.dt.bfloat16
AF = mybir.ActivationFunctionType
ALU = mybir.AluOpType

T = 2048          # tokens per core (one batch entry)
D = 1024          # model dim
HL = 4            # local heads per core
DH = 64           # head dim
CI = HL * DH      # local inner dim = 256
NT = T // 128     # 16 token tiles
NK = D // 128     # 8 dim chunks
LN_EPS = 1e-5
SCALE = DH ** -0.5

_NC_CACHE = {}


def _build():
    nc = bacc.Bacc("TRN2", target_bir_lowering=False, debug=False)

    x = nc.dram_tensor("x", [T, D], F32, kind="ExternalInput")
    wq = nc.dram_tensor("wq", [D, CI], BF16, kind="ExternalInput")
    wk = nc.dram_tensor("wk", [D, CI], BF16, kind="ExternalInput")
    wv = nc.dram_tensor("wv", [D, CI], BF16, kind="ExternalInput")
    wo = nc.dram_tensor("wo", [CI, D], BF16, kind="ExternalInput")
    out = nc.dram_tensor("out", [T, D], F32, kind="ExternalOutput")

    x_t = x.rearrange("(t p) d -> t p d", p=128)          # [16, 128, 1024]
    out_t = out.rearrange("(t p) d -> t p d", p=128)
    wq_t = wq.rearrange("(c p) n -> p c n", p=128)        # [128, 8, 256]
    wk_t = wk.rearrange("(c p) n -> p c n", p=128)
    wv_t = wv.rearrange("(c p) n -> p c n", p=128)
    wo_t = wo.rearrange("(c p) n -> p c n", p=128)        # [128, 2, 1024]

    with tile.TileContext(nc) as tc:
        with (
            tc.tile_pool(name="persist", bufs=1) as persist,
            tc.tile_pool(name="g_ps", bufs=1, space="PSUM") as g_ps,
        ):
            eps = persist.tile([128, 1], F32, name="eps")
            nc.vector.memset(eps, LN_EPS)
            ident_f = persist.tile([128, 128], F32, name="ident_f")
            make_identity(nc, ident_f)
            ident = persist.tile([128, 128], BF16, name="ident")
            nc.vector.tensor_copy(out=ident, in_=ident_f)

            # persistent activations / weights (all bf16)
            qkT = persist.tile([128, 4, T], BF16, name="qkT")        # 16KB/p
            vext = persist.tile([128, NT, HL, 65], BF16, name="vext")
            OT = persist.tile([128, 2, T], BF16, name="OT")          # 8KB/p
            wq_r = persist.tile([128, NK, CI], BF16, name="wq_r")
            wk_r = persist.tile([128, NK, CI], BF16, name="wk_r")
            wv_r = persist.tile([128, NK, CI], BF16, name="wv_r")
            wo_r = persist.tile([128, 2, D], BF16, name="wo_r")

            # ------------- Phase A/B: LN + transpose + QKV, ic-major -------------
            with (
                tc.tile_pool(name="ab_sb", bufs=3) as ab_sb,
                tc.tile_pool(name="ab_sm", bufs=8) as ab_sm,
            ):
                # x tile 0 split across both queues (fastest LN start);
                # weights (small in bf16) interleaved into x-stream slack;
                # deep xt buffering keeps both queues streaming
                xts = {}
                xt0 = ab_sb.tile([128, D], F32, tag="xt", name="xt", bufs=8)
                nc.sync.dma_start(xt0[:, 0:512], x_t[0][:, 0:512])
                nc.scalar.dma_start(xt0[:, 512:1024], x_t[0][:, 512:1024])
                xts[0] = xt0
                xt1 = ab_sb.tile([128, D], F32, tag="xt", name="xt", bufs=8)
                nc.scalar.dma_start(xt1[:, 0:512], x_t[1][:, 0:512])
                nc.sync.dma_start(xt1[:, 512:1024], x_t[1][:, 512:1024])
                xts[1] = xt1
                for tt in (3,):
                    xt3 = ab_sb.tile([128, D], F32, tag="xt", name="xt", bufs=8)
                    nc.scalar.dma_start(xt3, x_t[tt])
                    xts[tt] = xt3
                for tt in (2, 4):
                    xt2 = ab_sb.tile([128, D], F32, tag="xt", name="xt", bufs=8)
                    nc.sync.dma_start(xt2, x_t[tt])
                    xts[tt] = xt2
                nc.scalar.dma_start(wv_r, wv_t)
                nc.sync.dma_start(wq_r, wq_t)
                nc.sync.dma_start(wk_r, wk_t)

                # ones column of vext
                onev = ab_sm.tile([128, NT * HL], BF16, tag="onev")
                nc.vector.memset(onev, 1.0)
                nc.vector.tensor_copy(
                    out=vext[:, :, :, 64],
                    in_=onev.rearrange("p (t h) -> p t h", t=NT),
                )

                for ic in range(4):
                    # [d%128, tl, kc, t]: each tile's transpose lands in a
                    # contiguous [128, 8, 128] destination slice
                    xnT_ic = ab_sb.tile([128, 4, NK, 128], BF16, tag="xnTic",
                                        name="xnT_ic", bufs=2)
                    for tl in range(4):
                        tt = ic * 4 + tl
                        if tt in xts:
                            xt = xts.pop(tt)
                        else:
                            xt = ab_sb.tile([128, D], F32, tag="xt", name="xt",
                                            bufs=8)
                            (nc.scalar if tt % 2 else nc.sync).dma_start(
                                xt, x_t[tt])
                        stats = ab_sm.tile([128, 2, 6], F32, tag="stats",
                                           name="stats")
                        xr = xt.rearrange("p (c f) -> p c f", f=512)
                        for c in range(2):
                            nc.vector.bn_stats(out=stats[:, c, :], in_=xr[:, c, :])
                        mv = ab_sm.tile([128, 2], F32, tag="mv", name="mv")
                        nc.vector.bn_aggr(out=mv, in_=stats)
                        rstd = ab_sm.tile([128, 1], F32, tag="rstd", name="rstd")
                        nc.scalar.activation(out=rstd, in_=mv[:, 1:2], func=AF.Sqrt,
                                             bias=eps, scale=1.0)
                        nc.vector.reciprocal(out=rstd, in_=rstd)
                        xn = ab_sb.tile([128, D], BF16, tag="xn", name="xn",
                                        bufs=4)
                        nc.vector.tensor_scalar(out=xn, in0=xt, scalar1=mv[:, 0:1],
                                                scalar2=rstd, op0=ALU.subtract,
                                                op1=ALU.mult)
                        for kc4 in range(NK // 4):
                            pt = g_ps.tile([128, 4, 128], BF16, tag="b1",
                                           name="pt", bufs=4)
                            for q in range(4):
                                nc.tensor.transpose(
                                    pt[:, q, :],
                                    xn[:, (kc4 * 4 + q) * 128:(kc4 * 4 + q + 1) * 128],
                                    ident)
                            dst = xnT_ic[:, tl, kc4 * 4:kc4 * 4 + 4, :]
                            # the two stashes run concurrently on ACT+DVE so
                            # the tile's xnT (gating V/qk) is ready sooner
                            if kc4 == 0:
                                nc.scalar.activation(out=dst, in_=pt,
                                                     func=AF.Copy)
                            else:
                                nc.vector.tensor_copy(out=dst, in_=pt)

                    # v natural for these 4 token tiles
                    for tl in range(4):
                        pv = g_ps.tile([128, CI], F32, tag="b1", name="pv", bufs=4)
                        for kc in range(NK):
                            nc.tensor.matmul(
                                pv,
                                lhsT=xnT_ic[:, tl, kc, :],
                                rhs=wv_r[:, kc, :],
                                start=(kc == 0), stop=(kc == NK - 1))
                        nc.vector.tensor_copy(
                            out=vext[:, ic * 4 + tl, :, 0:64],
                            in_=pv.rearrange("p (h d) -> p h d", h=HL))

                    # qT/kT columns for this token chunk (rhs spans all 4
                    # token tiles via a strided AP)
                    sq = [g_ps.tile([128, 1024], F32, tag="s", name=f"sq{i}",
                                    bufs=2) for i in range(2)]
                    pq = [sq[i // 2][:, (i % 2) * 512:(i % 2 + 1) * 512]
                          for i in range(4)]
                    for kc in range(NK):
                        for pc in range(4):
                            w_src = wq_r if pc < 2 else wk_r
                            off = (pc % 2) * 128
                            nc.tensor.matmul(
                                pq[pc],
                                lhsT=w_src[:, kc, off:off + 128],
                                rhs=xnT_ic[:, :, kc, :],
                                start=(kc == 0), stop=(kc == NK - 1))
                    for pc in range(4):
                        dst = qkT[:, pc, ic * 512:(ic + 1) * 512]
                        if pc % 2 == 0:
                            nc.vector.tensor_copy(out=dst, in_=pq[pc])
                        else:
                            nc.scalar.activation(out=dst, in_=pq[pc],
                                                 func=AF.Copy)

                # wo only feeds the phase-D tail; DMA it behind the x tiles
                nc.scalar.dma_start(wo_r, wo_t)

            # ---------------- Phase C: attention, 4 passes ----------------
            with (
                tc.tile_pool(name="c_exp", bufs=3) as c_exp,
                tc.tile_pool(name="c_sm", bufs=8) as c_sm,
            ):
                # packed r rows per pass: [1, 4, 512] on one partition; one
                # fast approx reciprocal per pass at the next boundary
                rqs = [c_sm.tile([1, 4, 512], F32, tag="rq", name=f"rq{p}",
                                 bufs=1)
                       for p in range(4)]

                def drip_norm(p, pr, half):
                    """normalize pass p's OT slices: reciprocal (DVE) ->
                    partition_broadcast (GpSimd, SBUF only) -> mult (DVE).
                    Touches neither PSUM nor the PE."""
                    rq = rqs[p]
                    rqf = rq.rearrange("p a b -> p (a b)")
                    nc.vector.reciprocal_approx_fast(out=rqf, in_=rqf)
                    for k in range(4):
                        hp, i2 = k // 2, k % 2
                        ic = half * 2 + i2
                        po = hp * 64
                        rb = c_sm.tile([128, 512], F32, tag="rb", name="rb",
                                       bufs=4)
                        nc.gpsimd.partition_broadcast(rb, rq[0:1, k, :])
                        sl = OT[po:po + 64, pr, ic * 512:(ic + 1) * 512]
                        nc.vector.tensor_tensor(out=sl, in0=sl,
                                                in1=rb[po:po + 64, :],
                                                op=ALU.mult)

                # passes ordered so both chunks of a token half finish early
                passes = [(0, 0), (1, 0), (0, 1), (1, 1)]
                for p, (pr, half) in enumerate(passes):
                    qc = pr                      # chunk holding both heads' q
                    kcnk = 2 + pr                # chunk holding both heads' k
                    if p > 0:
                        drip_norm(p - 1, *passes[p - 1])
                    ps_o = [g_ps.tile([65, 512], F32, tag="b1",
                                      name=f"o{i}", bufs=4)
                            for i in range(4)]   # [head parity][i2]

                    prev = None
                    steps = [(jt, i2) for jt in range(NT) for i2 in range(2)]
                    for jt, i2 in steps:
                        ic = half * 2 + i2
                        # one psum tile: [head0 chunk | head1 chunk]
                        ps_s = g_ps.tile([128, 1024], F32, tag="s",
                                         name="ps_s", bufs=2)
                        for hp in range(2):
                            po = hp * 64
                            nc.tensor.matmul(
                                ps_s[:, hp * 512:(hp + 1) * 512],
                                lhsT=qkT[po:po + 64, kcnk,
                                         jt * 128:(jt + 1) * 128],
                                rhs=qkT[po:po + 64, qc,
                                        ic * 512:(ic + 1) * 512],
                                start=True, stop=True)
                        ex = c_exp.tile([128, 1024], BF16, tag="e",
                                        name="ex", bufs=8)
                        nc.scalar.activation(out=ex, in_=ps_s,
                                             func=AF.Exp, scale=SCALE)
                        # O matmuls lag one step so the PE never waits on ACT
                        if prev is not None:
                            pjt, pi2, pex = prev
                            for hp in range(2):
                                nc.tensor.matmul(
                                    ps_o[hp * 2 + pi2],
                                    lhsT=vext[:, pjt, pr * 2 + hp, :],
                                    rhs=pex[:, hp * 512:(hp + 1) * 512],
                                    start=(pjt == 0), stop=(pjt == NT - 1),
                                    skip_group_check=True)
                        prev = (jt, i2, ex)
                    pjt, pi2, pex = prev
                    for hp in range(2):
                        nc.tensor.matmul(
                            ps_o[hp * 2 + pi2],
                            lhsT=vext[:, pjt, pr * 2 + hp, :],
                            rhs=pex[:, hp * 512:(hp + 1) * 512],
                            start=(pjt == 0), stop=(pjt == NT - 1),
                            skip_group_check=True)

                    # stash r rows + unnormalized O^T (psum->sbuf, DVE)
                    for i2 in range(2):
                        for hp in range(2):
                            ic = half * 2 + i2
                            po = hp * 64
                            nc.vector.tensor_copy(
                                out=rqs[p][0:1, hp * 2 + i2, :],
                                in_=ps_o[hp * 2 + i2][64:65, :])
                            nc.vector.tensor_copy(
                                out=OT[po:po + 64, qc,
                                       ic * 512:(ic + 1) * 512],
                                in_=ps_o[hp * 2 + i2][0:64, :])

                # ---------------- Phase D: tail ----------------
                def d_one(tt, ncn):
                    pd = g_ps.tile([128, 512], F32, tag="b1", name="pd",
                                   bufs=4)
                    for ck in range(2):
                        nc.tensor.matmul(
                            pd,
                            lhsT=OT[:, ck, tt * 128:(tt + 1) * 128],
                            rhs=wo_r[:, ck, ncn * 512:(ncn + 1) * 512],
                            start=(ck == 0), stop=(ck == 1))
                    ot = c_exp.tile([128, 512], F32, tag="ot", name="ot",
                                    bufs=6)
                    if (tt * 2 + ncn) % 2 == 0:
                        nc.scalar.activation(out=ot, in_=pd, func=AF.Copy)
                    else:
                        nc.vector.tensor_copy(out=ot, in_=pd)
                    (nc.sync if (tt * 2 + ncn) % 2 == 0
                     else nc.scalar).dma_start(
                        out_t[tt][:, ncn * 512:(ncn + 1) * 512], ot)

                # first token half doesn't depend on the last pass's norm
                for tt in range(NT // 2):
                    d_one(tt, 0)
                    d_one(tt, 1)
                drip_norm(3, *passes[3])
                for tt in range(NT // 2, NT):
                    d_one(tt, 0)
                    d_one(tt, 1)

    nc.compile()
    return nc


def make_in_maps(x, gamma, beta, w_qkv, w_out, b_out):
    """Shard full inputs into the 8 per-core input maps (batch x head-group).
    Weights are pre-converted to bf16 on the host (matmul precision)."""
    x = np.asarray(x, dtype=np.float32)
    gamma = np.asarray(gamma, dtype=np.float32)
    w_qkv = np.asarray(w_qkv, dtype=np.float32)
    w_out = np.asarray(w_out, dtype=np.float32)

    wg = (w_qkv * gamma[:, None]).astype(ml_dtypes.bfloat16)
    wo16 = w_out.astype(ml_dtypes.bfloat16)
    in_maps = []
    for core in range(8):
        b, g = core // 4, core % 4
        cs = slice(g * CI, (g + 1) * CI)
        in_maps.append({
            "x": np.ascontiguousarray(x[b]),
            "wq": np.ascontiguousarray(wg[:, 0 * 1024:1 * 1024][:, cs]),
            "wk": np.ascontiguousarray(wg[:, 1 * 1024:2 * 1024][:, cs]),
            "wv": np.ascontiguousarray(wg[:, 2 * 1024:3 * 1024][:, cs]),
            "wo": np.ascontiguousarray(wo16[cs, :]),
        })
    return in_maps


def kernel(x, gamma, beta, w_qkv, w_out, b_out):
    """Full inputs in, full output out.  Shards batch x head-groups over 8
    cores, runs the SPMD Bass kernel, and sums the partial projections."""
    if "nc" not in _NC_CACHE:
        _NC_CACHE["nc"] = _build()
    nc = _NC_CACHE["nc"]

    b_out = np.asarray(b_out, dtype=np.float32)
    in_maps = make_in_maps(x, gamma, beta, w_qkv, w_out, b_out)

    res = bass_utils.run_bass_kernel_spmd(nc, in_maps, core_ids=list(range(8)))
    parts = [r["out"] for r in res.results]
    full = np.stack([
        parts[0] + parts[1] + parts[2] + parts[3],
        parts[4] + parts[5] + parts[6] + parts[7],
    ]).astype(np.float32)
    return full + b_out
